# revision 1
# baseline (speedup 1.0000x reference)
"""DetConB loss (nn_DetConBLoss) on 8 TRN2 NeuronCores via Bass/Tile.

Strategy (data-parallel over batch, targets replicated):
  - Host: l2-normalize preds/targets in f32, flatten to (4096, 256),
    transpose to (d, rows), cast bf16. Core c owns pred rows
    [c*512, (c+1)*512). Each core receives the full targets with columns
    rolled by c*512 so its own-image diagonal band sits at a fixed,
    compile-time-constant column range (the program is SPMD-identical).
  - Device (per core): for each of the 4 pred x target combinations,
    a (512 x 4096) fp8 DoubleRow matmul (K=256 in one pass, fp32 PSUM
    accum) fused with exp(scale*x) on ScalarE at its roofline; row-sums
    via ACTIVATE's accumulator on one PSUM buffer and a DVE reduce on
    the other. Only the 32 KB of row-sum partials leave the device.
  - Host: the 16x16 own-image diagonal dot blocks (recomputed from the
    same fp8 inputs, ~0.4% of total FLOPs), masks from the roi indices,
    positive-pair sums, the -inf masking correction (subtract the exp of
    masked entries from the denominators), log, and the final mean.

All 34.4 GFLOP of matmul and the 67M-element exp run on device; the host
handles O(b*n^2)-scale arithmetic.
"""
import numpy as np
import ml_dtypes

import concourse.bacc as bacc
import concourse.mybir as mybir
import concourse.tile as tile
from concourse.bass_utils import run_bass_kernel_spmd

TEMP = 0.1
EPS = 1e-11
SCALE = float(np.float32(1.0 / (TEMP + EPS)))
NCORES = 8
B, N, D = 256, 16, 256
R = B * N          # 4096 flat rows
RPC = R // NCORES  # 512 rows per core
MT = RPC // 128    # 4 row-tiles of 128 per core
BF16 = mybir.dt.bfloat16
FP8 = mybir.dt.float8e4
NPFP8 = ml_dtypes.float8_e4m3
F32 = mybir.dt.float32
I32 = mybir.dt.int32
# Schraudolph fast-exp: exp(s*x) ~= bitcast_f32(int32(x*SA + SB))
SA = float(np.float32((2**23 / np.log(2.0)) * (1.0 / (0.1 + 1e-11))))
SB = float(np.float32(127 * 2**23 - 486411))


def build_nc():
    """Build + schedule + compile the SPMD per-core Bass program."""
    nc = bacc.Bacc("TRN2", target_bir_lowering=False, debug=False,
                   num_devices=NCORES)

    p_dram = [nc.dram_tensor(f"p{i + 1}t", [D, RPC], FP8, kind="ExternalInput")
              for i in range(2)]
    t_dram = [nc.dram_tensor(f"t{i + 1}t", [D, R], FP8, kind="ExternalInput")
              for i in range(2)]
    sacc = nc.dram_tensor("sacc", [128, 80], F32, kind="ExternalOutput")

    with tile.TileContext(nc) as tc:
        with (
            tc.tile_pool(name="const", bufs=1) as const_pool,
            tc.tile_pool(name="psum", bufs=2, space="PSUM") as psum_pool,
            tc.tile_pool(name="scratch", bufs=6) as scratch_pool,
        ):
            # Persistent SBUF: targets as [K=128 partitions, kchunk*R + col],
            # preds as [128, kchunk*RPC + col].
            t_sb = [const_pool.tile([128, 2 * R], FP8, name=f"t_sb{i}", tag=f"t{i}")
                    for i in range(2)]
            p_sb = [const_pool.tile([128, 2 * RPC], FP8, name=f"p_sb{i}", tag=f"p{i}")
                    for i in range(2)]

            # All 32 row-sum partials live in one persistent strip; a single
            # 32 KB DMA ships them at the end (col 2*it = g0 via DVE reduce,
            # col 2*it+1 = g1 via ACT accumulator).
            strip = const_pool.tile([128, 80], F32, name="strip", tag="strip")
            nc.vector.memset(strip, 0.0)
            # Explicit zero-bias AP: a float bias would be lowered through the
            # const-AP machinery, whose TENSOR_LOAD sits in the preamble.
            zbias = const_pool.tile([128, 1], F32, name="zbias", tag="zbias")
            nc.vector.memset(zbias, 0.0)
            # Warm the exp table set during the input-DMA window so the first
            # real ACTIVATE does not pay the ~2.7us ACT_TABLE_LOAD.
            nc.scalar.activation(strip[:, 0:2], strip[:, 0:2],
                                 mybir.ActivationFunctionType.Exp, bias=zbias)
            nc.vector.memset(strip[:, 0:2], 0.0)

            # Input DMAs on the sync (HWDGE) queue, ordered by first use:
            # p1 + the first two t1 chunks gate iteration 0.
            def load_t(tsel, k, g):
                cs = g * 2048
                nc.sync.dma_start(
                    out=t_sb[tsel][:, k * R + cs: k * R + cs + 2048],
                    in_=t_dram[tsel][k * 128:(k + 1) * 128, cs:cs + 2048])

            def load_p(px):
                nc.sync.dma_start(
                    out=p_sb[px].rearrange("p (k c) -> p k c", k=2),
                    in_=p_dram[px].ap().rearrange("(k p) c -> p k c", p=128))

            def load_t_fine(tsel, k, g, q):
                cs = g * 2048 + q * 1024
                nc.sync.dma_start(
                    out=t_sb[tsel][:, k * R + cs: k * R + cs + 1024],
                    in_=t_dram[tsel][k * 128:(k + 1) * 128, cs:cs + 1024])

            load_p(0)
            for q in range(2):
                load_t_fine(0, 0, 0, q)
                load_t_fine(0, 1, 0, q)
            load_t(0, 0, 1)
            load_t(0, 1, 1)
            load_p(1)
            for k in range(2):
                for g in range(2):
                    load_t(1, k, g)

            # tsel outer: the first 8 halves consume only t1, so the t2
            # load (2 MB) hides behind ~28 us of compute.
            for tsel in range(2):
                for px in range(2):
                    for mt in range(MT):
                        it = tsel * 8 + px * MT + mt
                        # One 4096-col half = both PSUM buffers. k-outer so 8
                        # consecutive matmuls share the stationary weights and
                        # stream back-to-back (no LDWEIGHTS-induced drain).
                        ps = [psum_pool.tile([128, 2048], F32, name=f"ps{h}",
                                             tag="ps")
                              for h in range(2)]
                        # fp8 DoubleRow: both 128-deep K chunks contract in a
                        # single pass (lhsT/rhs carry the k pair on a middle
                        # AP dim), so each 512-col tile is one matmul.
                        lhs3 = p_sb[px].rearrange("p (k c) -> p k c", k=2)
                        rhs3 = t_sb[tsel].rearrange("p (k c) -> p k c", k=2)
                        for g in range(2):
                            for j in range(4):
                                c0 = g * 2048 + j * 512
                                nc.tensor.matmul(
                                    ps[g][:, j * 512:(j + 1) * 512],
                                    lhs3[:, :, mt * 128:(mt + 1) * 128],
                                    rhs3[:, :, c0:c0 + 512],
                                    start=True, stop=True,
                                    perf_mode=mybir.MatmulPerfMode.DoubleRow)
                        # (The own-image diagonal blocks are recomputed on
                        # the host from the same fp8 inputs — no band output.)
                        # g0: ACT accumulator (its READ_ACCUMULATOR lands
                        # mid-period, off the inter-iteration critical path);
                        # g1: exp on ACT, row-sum on the otherwise-idle DVE.
                        scr0 = scratch_pool.tile([128, 2048], BF16, name="scr0",
                                                 tag="scr")
                        nc.scalar.activation(
                            scr0, ps[0], mybir.ActivationFunctionType.Exp,
                            bias=zbias, scale=SCALE,
                            accum_out=strip[:, 2 * it:2 * it + 1])
                        scr1 = scratch_pool.tile([128, 2048], BF16, name="scr1",
                                                 tag="scr")
                        if it == 15:
                            # Final iteration: DVE work would sit on the
                            # kernel-exit path; the ACT accumulator's read-out
                            # is cheaper there.
                            nc.scalar.activation(
                                scr1, ps[1], mybir.ActivationFunctionType.Exp,
                                bias=zbias, scale=SCALE,
                                accum_out=strip[:, 2 * it + 1:2 * it + 2])
                        else:
                            # Half of g1 goes through a Schraudolph fast-exp
                            # on the now-idle DVE (int-converting multiply-add
                            # + reduce of the bitcast), shortening the
                            # critical ScalarE chain to 2048+1024 columns.
                            sch = scratch_pool.tile([128, 1024], I32,
                                                    name="sch", tag="sch")
                            nc.vector.tensor_scalar(
                                sch, ps[1][:, 1024:2048], SA, SB,
                                op0=mybir.AluOpType.mult,
                                op1=mybir.AluOpType.add)
                            nc.vector.tensor_reduce(
                                strip[:, 64 + it:65 + it], sch.bitcast(F32),
                                axis=mybir.AxisListType.X, op=mybir.AluOpType.add)
                            nc.scalar.activation(
                                scr1[:, 0:1024], ps[1][:, 0:1024],
                                mybir.ActivationFunctionType.Exp,
                                bias=zbias, scale=SCALE)
                            nc.vector.tensor_reduce(
                                strip[:, 2 * it + 1:2 * it + 2], scr1[:, 0:1024],
                                axis=mybir.AxisListType.X, op=mybir.AluOpType.add)
            # Final strip DMA on the sync HWDGE queue: the gpsimd SWDGE
            # drain at kernel exit is ~2.4us when it must wait for this
            # transfer; HWDGE drains in ~0.1us.
            nc.sync.dma_start(out=sacc.ap(), in_=strip)

    nc.compile()
    return nc


_NC = None


def _get_nc():
    global _NC
    if _NC is None:
        _NC = build_nc()
    return _NC


def _l2norm(x):
    return x / np.linalg.norm(x, axis=-1, keepdims=True)


def host_prep(pred1, pred2, target1, target2):
    p1t = _l2norm(np.asarray(pred1, np.float32)).reshape(R, D).T.astype(NPFP8)
    p2t = _l2norm(np.asarray(pred2, np.float32)).reshape(R, D).T.astype(NPFP8)
    t1t = _l2norm(np.asarray(target1, np.float32)).reshape(R, D).T.astype(NPFP8)
    t2t = _l2norm(np.asarray(target2, np.float32)).reshape(R, D).T.astype(NPFP8)
    # Raw own-image diagonal dot blocks (b, n, m), fp8-quantized operands in
    # f32 — the same products the device computes, ~0.4% of total FLOPs.
    pf = [p1t.T.astype(np.float32).reshape(B, N, D),
          p2t.T.astype(np.float32).reshape(B, N, D)]
    tf = [t1t.T.astype(np.float32).reshape(B, N, D),
          t2t.T.astype(np.float32).reshape(B, N, D)]
    diag = [[np.einsum('bnd,bmd->bnm', pf[px], tf[ts]).astype(np.float32)
             for ts in range(2)] for px in range(2)]
    in_maps = []
    for c in range(NCORES):
        r0 = c * RPC
        in_maps.append({
            "p1t": np.ascontiguousarray(p1t[:, r0:r0 + RPC]),
            "p2t": np.ascontiguousarray(p2t[:, r0:r0 + RPC]),
            "t1t": np.ascontiguousarray(np.concatenate([t1t[:, r0:], t1t[:, :r0]], axis=1)),
            "t2t": np.ascontiguousarray(np.concatenate([t2t[:, r0:], t2t[:, :r0]], axis=1)),
        })
    return in_maps, diag


def host_post(results, diag, pind1, pind2, tind1, tind2):
    S = np.zeros((2, R), np.float64)
    for c, res in enumerate(results):
        sacc = np.asarray(res["sacc"])
        for px in range(2):
            for mt in range(MT):
                r0 = c * RPC + mt * 128
                cols = [2 * (tsel * 8 + px * MT + mt) + g
                        for tsel in range(2) for g in range(2)]
                cols += [64 + tsel * 8 + px * MT + mt for tsel in range(2)]
                S[px, r0:r0 + 128] = sacc[:, cols].astype(np.float64).sum(axis=1)
    sc = np.float32(SCALE)
    D_aa = sc * diag[0][0]
    D_ab = sc * diag[0][1]
    D_ba = sc * diag[1][0]
    D_bb = sc * diag[1][1]

    f32 = np.float32
    pind1, pind2 = np.asarray(pind1), np.asarray(pind2)
    tind1, tind2 = np.asarray(tind1), np.asarray(tind2)
    same_aa = (pind1[:, :, None] == tind1[:, None, :]).astype(f32)
    same_ab = (pind1[:, :, None] == tind2[:, None, :]).astype(f32)
    same_ba = (pind2[:, :, None] == tind1[:, None, :]).astype(f32)
    same_bb = (pind2[:, :, None] == tind2[:, None, :]).astype(f32)

    S0 = S[0].reshape(B, N)
    S1 = S[1].reshape(B, N)
    corr0 = (same_aa * np.exp(D_aa.astype(np.float64))).sum(-1)
    corr1 = (same_bb * np.exp(D_bb.astype(np.float64))).sum(-1)
    lse0 = np.log(S0 - corr0)
    lse1 = np.log(S1 - corr1)

    num_pos0 = same_ab.sum(-1)
    num_pos1 = same_ba.sum(-1)
    pos_sum0 = (same_ab * D_ab).sum(-1)
    pos_sum1 = (same_ba * D_ba).sum(-1)

    area0 = (pind1[:, :, None] == pind1[:, None, :]).astype(f32).sum(-1)
    area1 = (pind2[:, :, None] == pind2[:, None, :]).astype(f32).sum(-1)
    w0 = (num_pos0 > 0.001).astype(f32) / area0
    w1 = (num_pos1 > 0.001).astype(f32) / area1

    ce0 = -w0 * (pos_sum0 - num_pos0 * lse0) / np.maximum(num_pos0, 1.0)
    ce1 = -w1 * (pos_sum1 - num_pos1 * lse1) / np.maximum(num_pos1, 1.0)
    return np.float32(ce0.mean() + ce1.mean())


def run_hw(inputs, trace=False):
    nc = _get_nc()
    in_maps, diag = host_prep(inputs["pred1"], inputs["pred2"],
                              inputs["target1"], inputs["target2"])
    last_err = None
    for attempt in range(3):
        try:
            res = run_bass_kernel_spmd(nc, in_maps,
                                       core_ids=list(range(NCORES)),
                                       trace=trace)
            break
        except Exception as e:  # transient NRT device errors recover on retry
            last_err = e
            import time
            time.sleep(20 * (attempt + 1))
    else:
        raise last_err
    loss = host_post(res.results, diag, inputs["pind1"], inputs["pind2"],
                     inputs["tind1"], inputs["tind2"])
    return loss, res


def kernel(**inputs):
    loss, _ = run_hw(inputs, trace=False)
    return loss



# revision 3
# speedup vs baseline: 1.1130x; 1.1130x over previous
"""DetConB loss (nn_DetConBLoss) on 8 TRN2 NeuronCores via Bass/Tile.

Strategy (data-parallel over batch, targets replicated):
  - Host: l2-normalize preds/targets in f32, flatten to (4096, 256),
    transpose to (d, rows), cast fp8, k-interleave per 512-col block so
    every DMA is contiguous and every DoubleRow matmul AP is a dense 3D
    slice. Core c owns pred rows [c*512, (c+1)*512); targets arrive with
    columns rolled by c*512 (SPMD-identical program).
  - Device (per core): 16 iterations (tsel, px, mt) x 2 half-tiles of
    (128 rows x 2048 target cols). Per half: 4 fp8 DoubleRow matmuls
    (K=256 in one pass) into a double-buffered PSUM tile; the row-sum of
    exp(scale*logits) is split between ScalarE (ACTIVATE exp on cols
    0:1280 with its free accumulator) and VectorE (Schraudolph fast-exp
    int32 bit-trick on cols 1280:2048, then a fused
    scalar_tensor_tensor fold+accumulate over the staged halves). Only
    the 32 KB strip of row-sum partials leaves the device.
  - Host: 16x16 own-image diagonal dot blocks (recomputed from the same
    fp8 inputs, ~0.4% of FLOPs), roi masks, positive-pair sums, the
    -inf masking correction, log, and the final mean.
"""
import numpy as np
import ml_dtypes

import concourse.bacc as bacc
import concourse.mybir as mybir
import concourse.tile as tile
from concourse.bass_utils import run_bass_kernel_spmd

TEMP = 0.1
EPS = 1e-11
SCALE = float(np.float32(1.0 / (TEMP + EPS)))
NCORES = 8
B, N, D = 256, 16, 256
R = B * N          # 4096 flat rows
RPC = R // NCORES  # 512 rows per core
BF16 = mybir.dt.bfloat16
FP8 = mybir.dt.float8e4
NPFP8 = ml_dtypes.float8_e4m3
F32 = mybir.dt.float32
I32 = mybir.dt.int32
# Schraudolph fast-exp: exp(s*x) ~= bitcast_f32(int32(x*SA + SB))
SA = float(np.float32((2**23 / np.log(2.0)) * (1.0 / (0.1 + 1e-11))))
SB = float(np.float32(127 * 2**23 - 486411))

XA = 1280          # cols per half handled by ACT exp+accum
XZ = 2048 - XA     # cols per half handled by DVE schraudolph
NH = 2             # halves per iteration


def build_nc():
    """Build + schedule + compile the SPMD per-core Bass program."""
    nc = bacc.Bacc("TRN2", target_bir_lowering=False, debug=False,
                   num_devices=NCORES)

    # k-interleaved layouts: p [128, mt(4) * k(2) * 128], t [128, blk(8) *
    # k(2) * 512] fp8.
    p_dram = [nc.dram_tensor(f"p{i + 1}t", [128, 1024], FP8,
                             kind="ExternalInput") for i in range(2)]
    t_dram = [nc.dram_tensor(f"t{i + 1}t", [128, 8192], FP8,
                             kind="ExternalInput") for i in range(2)]
    sacc = nc.dram_tensor("sacc", [128, 64], F32, kind="ExternalOutput")

    with tile.TileContext(nc) as tc:
        with (
            tc.tile_pool(name="const", bufs=1) as const_pool,
            tc.tile_pool(name="psum", bufs=2, space="PSUM") as psum_pool,
            tc.tile_pool(name="scr", bufs=2) as scr_pool,
            tc.tile_pool(name="stage", bufs=2) as stage_pool,
        ):
            t_sb = [const_pool.tile([128, 8192], FP8, name=f"t_sb{i}",
                                    tag=f"t{i}") for i in range(2)]
            p_sb = [const_pool.tile([128, 1024], FP8, name=f"p_sb{i}",
                                    tag=f"p{i}") for i in range(2)]
            strip = const_pool.tile([128, 64], F32, name="strip", tag="strip")
            zbias = const_pool.tile([128, 1], F32, name="zbias", tag="zbias")
            warm = const_pool.tile([128, 2], BF16, name="warm", tag="warm")
            nc.vector.memset(zbias, 0.0)
            nc.vector.memset(warm, 0.0)
            # Warm the exp table set (ACT queue) before the first real
            # ACTIVATE so it does not pay the ~2.7us ACT_TABLE_LOAD.
            nc.scalar.activation(warm, warm,
                                 mybir.ActivationFunctionType.Exp,
                                 bias=zbias)

            # Input DMAs. sync (HWDGE) carries the critical path: p1 and
            # t1; scalar (also HWDGE on TRN2) carries p2 + the first t2
            # chunk; remaining t2 chunks are issued inside the loop from
            # the scalar queue to fill its idle slots.
            nc.sync.dma_start(out=p_sb[0], in_=p_dram[0].ap())
            nc.sync.dma_start(out=t_sb[0][:, 0:1024],
                              in_=t_dram[0][:, 0:1024])
            nc.sync.dma_start(out=t_sb[0][:, 1024:2048],
                              in_=t_dram[0][:, 1024:2048])
            nc.sync.dma_start(out=t_sb[0][:, 2048:4096],
                              in_=t_dram[0][:, 2048:4096])
            nc.sync.dma_start(out=t_sb[0][:, 4096:6144],
                              in_=t_dram[0][:, 4096:6144])
            nc.sync.dma_start(out=t_sb[0][:, 6144:8192],
                              in_=t_dram[0][:, 6144:8192])
            nc.scalar.dma_start(out=p_sb[1], in_=p_dram[1].ap())
            # t2 chunks deferred into the loop (4 x 2048 fp8 cols each).
            t2_chunks = [(i * 2048, (i + 1) * 2048) for i in range(4)]

            for tsel in range(2):
                for px in range(2):
                    for mt in range(4):
                        it = tsel * 8 + px * 4 + mt
                        lhs = p_sb[px][:, mt * 256:(mt + 1) * 256].rearrange(
                            "p (k c) -> p k c", k=2)
                        for h in range(2):
                            ps = psum_pool.tile([128, 2048], F32, name="ps",
                                                tag="ps")
                            for jj in range(4):
                                blk = 4 * h + jj
                                rhs = t_sb[tsel][
                                    :, blk * 1024:(blk + 1) * 1024
                                ].rearrange("p (k c) -> p k c", k=2)
                                nc.tensor.matmul(
                                    ps[:, jj * 512:(jj + 1) * 512], lhs, rhs,
                                    start=True, stop=True,
                                    perf_mode=mybir.MatmulPerfMode.DoubleRow)
                            c0 = 4 * it + 2 * h
                            scr = scr_pool.tile([128, XA], BF16, name="scr",
                                                tag="scr")
                            nc.scalar.activation(
                                scr, ps[:, 0:XA],
                                mybir.ActivationFunctionType.Exp,
                                bias=zbias, scale=SCALE,
                                accum_out=strip[:, c0:c0 + 1])
                            stage = stage_pool.tile([128, XZ], I32,
                                                    name="stage", tag="stg")
                            nc.vector.tensor_scalar(
                                stage, ps[:, XA:2048], SA, SB,
                                op0=mybir.AluOpType.mult,
                                op1=mybir.AluOpType.add)
                            stf = stage.bitcast(F32)
                            dum = scr_pool.tile([128, XZ // 2], F32,
                                                name="dum", tag="dum")
                            nc.vector.scalar_tensor_tensor(
                                dum, stf[:, 0:XZ // 2], 1.0,
                                stf[:, XZ // 2:XZ],
                                op0=mybir.AluOpType.mult,
                                op1=mybir.AluOpType.add,
                                accum_out=strip[:, c0 + 1:c0 + 2])
                        if tsel == 0 and px == 0 and t2_chunks:
                            a, b = t2_chunks.pop(0)
                            nc.scalar.dma_start(out=t_sb[1][:, a:b],
                                                in_=t_dram[1][:, a:b])
            nc.sync.dma_start(out=sacc.ap(), in_=strip)

    nc.compile()
    return nc


_NC = None


def _get_nc():
    global _NC
    if _NC is None:
        _NC = build_nc()
    return _NC


def _l2norm(x):
    return x / np.linalg.norm(x, axis=-1, keepdims=True)


def _swizzle_p(pt):
    """[D=256, 512] fp8 -> [128, mt(4) x k(2) x 128] contiguous."""
    return np.ascontiguousarray(
        pt.reshape(2, 128, 4, 128).transpose(1, 2, 0, 3).reshape(128, 1024))


def _swizzle_t(tt):
    """[D=256, 4096] fp8 -> [128, blk(8) x k(2) x 512] contiguous."""
    return np.ascontiguousarray(
        tt.reshape(2, 128, 8, 512).transpose(1, 2, 0, 3).reshape(128, 8192))


def host_prep(pred1, pred2, target1, target2):
    p1t = _l2norm(np.asarray(pred1, np.float32)).reshape(R, D).T.astype(NPFP8)
    p2t = _l2norm(np.asarray(pred2, np.float32)).reshape(R, D).T.astype(NPFP8)
    t1t = _l2norm(np.asarray(target1, np.float32)).reshape(R, D).T.astype(NPFP8)
    t2t = _l2norm(np.asarray(target2, np.float32)).reshape(R, D).T.astype(NPFP8)
    # Raw own-image diagonal dot blocks (b, n, m), fp8-quantized operands in
    # f32 — the same products the device computes, ~0.4% of total FLOPs.
    pf = [p1t.T.astype(np.float32).reshape(B, N, D),
          p2t.T.astype(np.float32).reshape(B, N, D)]
    tf = [t1t.T.astype(np.float32).reshape(B, N, D),
          t2t.T.astype(np.float32).reshape(B, N, D)]
    diag = [[np.einsum('bnd,bmd->bnm', pf[px], tf[ts]).astype(np.float32)
             for ts in range(2)] for px in range(2)]
    in_maps = []
    for c in range(NCORES):
        r0 = c * RPC
        t1r = np.concatenate([t1t[:, r0:], t1t[:, :r0]], axis=1)
        t2r = np.concatenate([t2t[:, r0:], t2t[:, :r0]], axis=1)
        in_maps.append({
            "p1t": _swizzle_p(p1t[:, r0:r0 + RPC]),
            "p2t": _swizzle_p(p2t[:, r0:r0 + RPC]),
            "t1t": _swizzle_t(t1r),
            "t2t": _swizzle_t(t2r),
        })
    return in_maps, diag


def host_post(results, diag, pind1, pind2, tind1, tind2):
    S = np.zeros((2, R), np.float64)
    for c, res in enumerate(results):
        sacc = np.asarray(res["sacc"])
        for px in range(2):
            for mt in range(4):
                r0 = c * RPC + mt * 128
                cols = [4 * (tsel * 8 + px * 4 + mt) + j
                        for tsel in range(2) for j in range(4)]
                S[px, r0:r0 + 128] = sacc[:, cols].astype(np.float64).sum(axis=1)
    sc = np.float32(SCALE)
    D_aa = sc * diag[0][0]
    D_ab = sc * diag[0][1]
    D_ba = sc * diag[1][0]
    D_bb = sc * diag[1][1]

    f32 = np.float32
    pind1, pind2 = np.asarray(pind1), np.asarray(pind2)
    tind1, tind2 = np.asarray(tind1), np.asarray(tind2)
    same_aa = (pind1[:, :, None] == tind1[:, None, :]).astype(f32)
    same_ab = (pind1[:, :, None] == tind2[:, None, :]).astype(f32)
    same_ba = (pind2[:, :, None] == tind1[:, None, :]).astype(f32)
    same_bb = (pind2[:, :, None] == tind2[:, None, :]).astype(f32)

    S0 = S[0].reshape(B, N)
    S1 = S[1].reshape(B, N)
    corr0 = (same_aa * np.exp(D_aa.astype(np.float64))).sum(-1)
    corr1 = (same_bb * np.exp(D_bb.astype(np.float64))).sum(-1)
    lse0 = np.log(S0 - corr0)
    lse1 = np.log(S1 - corr1)

    num_pos0 = same_ab.sum(-1)
    num_pos1 = same_ba.sum(-1)
    pos_sum0 = (same_ab * D_ab).sum(-1)
    pos_sum1 = (same_ba * D_ba).sum(-1)

    area0 = (pind1[:, :, None] == pind1[:, None, :]).astype(f32).sum(-1)
    area1 = (pind2[:, :, None] == pind2[:, None, :]).astype(f32).sum(-1)
    w0 = (num_pos0 > 0.001).astype(f32) / area0
    w1 = (num_pos1 > 0.001).astype(f32) / area1

    ce0 = -w0 * (pos_sum0 - num_pos0 * lse0) / np.maximum(num_pos0, 1.0)
    ce1 = -w1 * (pos_sum1 - num_pos1 * lse1) / np.maximum(num_pos1, 1.0)
    return np.float32(ce0.mean() + ce1.mean())


def run_hw(inputs, trace=False):
    nc = _get_nc()
    in_maps, diag = host_prep(inputs["pred1"], inputs["pred2"],
                              inputs["target1"], inputs["target2"])
    last_err = None
    for attempt in range(3):
        try:
            res = run_bass_kernel_spmd(nc, in_maps,
                                       core_ids=list(range(NCORES)),
                                       trace=trace)
            break
        except Exception as e:  # transient NRT device errors recover on retry
            last_err = e
            import time
            time.sleep(20 * (attempt + 1))
    else:
        raise last_err
    loss = host_post(res.results, diag, inputs["pind1"], inputs["pind2"],
                     inputs["tind1"], inputs["tind2"])
    return loss, res


def kernel(**inputs):
    loss, _ = run_hw(inputs, trace=False)
    return loss
